# revision 70
# baseline (speedup 1.0000x reference)
"""Trainium2 Bass kernel for BinarizeConv2dSDP.

Math (reference):
    s   = M + rv @ Z          (the rsqrt normalization is sign-preserving:
                               w = (m + rv@z) * rsqrt(...) with rsqrt > 0,
                               so sign(w) == sign(s))
    bw  = sign(s)             (O, I, 3, 3)
    ba  = sign(x)             (B, C, H, W)
    out = conv2d(ba, bw, stride 1, pad 1) * Alpha

Strategy:
    - Data-parallel over batch: 8 cores x 4 images each. M/Z/Alpha replicated.
    - Weight synthesis on the PE: per 288-col chunk, one contiguous PSUM
      accumulation group [I @ M, then rv_k I @ Z_k x5] of fp32r matmuls at
      full rate (N >= 256); ACT signs straight from PSUM, PE transposes the
      signed taps into lhsT layout (lag-one behind the group signs), DVE
      packs to fp8. Only Z sees fp32r input rounding -> a handful of
      borderline sign flips, well under the 2e-2 gate.
      HW rules learned the hard way: accumulation groups must stay
      contiguous on the PE queue (no other matmul interleaved), and an
      engine write to PSUM followed by matmul accumulation races; only
      standalone LDWEIGHTS may fill arrival gaps (HAM clock-gate warm-up,
      along with filler matmuls before the stream).
    - Head DMAs on one sync queue, few big transfers (the trigger rate
      otherwise starves the wire; a second queue halves per-queue rate):
      x0's first row-chunk leads to absorb the queue's slow ramp, then M,
      Z as L(864)/R(288) column halves so chunk groups c0-c2 finish while
      the R half streams, then the rest of x. Image-0 sign chunks slot
      between the weight signs on ACT's FIFO.
    - Binarized conv, 5 passes per 8-row tile (was 6): sign(x) lives in a
      zero-padded [128, 115 x 64] fp8 image; rows 59..114 hold a one-column-
      left-shifted duplicate of rows 2..57. Vertical tap pairs (ky 0+1) use
      DoubleRow with rhs pair step 64 and lhsT pair stride 3*O; the
      (2,0)+(2,1) pair uses DoubleRow into the duplicate (rhs pair step
      57*64); (2,2) is a single matmul. +-1 is exact in fp8/bf16 and PSUM
      accumulates f32 -> integer-exact.
    - Evacuation (x Alpha) on DVE; output stored fp16 (conv integers
      <= 1152 are fp16-exact; only the Alpha product rounds, ~5e-4), host
      upcasts. Stores alternate sync/gpsimd queues, tail on sync.
    - BASS_KERNEL_CC=1 enables a sharded-synthesis variant (each core
      synthesizes 16 in-channels, AllGather rebuilds the lhsT): correct,
      but the collective eats the full SPMD core-launch skew (~50us on
      this stack), so it stays off.
"""

import os
import numpy as np

import concourse.bass as bass
import concourse.tile as tile
from concourse import bacc, mybir
from concourse.bass_utils import run_bass_kernel_spmd
from concourse.masks import make_identity

F32 = mybir.dt.float32
F32R = mybir.dt.float32r
F16 = mybir.dt.float16
BF16 = mybir.dt.bfloat16
FP8 = mybir.dt.float8e4

USE_FP8 = bool(int(os.environ.get("BASS_KERNEL_FP8", "1")))
OUT16 = bool(int(os.environ.get("BASS_KERNEL_OUT16", "1")))
N_WARM = int(os.environ.get("BASS_KERNEL_WARM", "12"))
# sharded weight synthesis + AllGather: correct, but the collective pays
# the full SPMD core-launch skew (~50us on this stack) -> off by default
CC = bool(int(os.environ.get("BASS_KERNEL_CC", "0"))) and USE_FP8

B_FULL = 32
N_CORES = 8
B_CORE = B_FULL // N_CORES  # 4 images per core
C = 128      # in channels
O = 128      # out channels
H = W = 56
HP = 58                      # padded rows
WP = 64 if USE_FP8 else 58   # padded row stride (64 -> vertical pair step 64B)
DUP = 57                     # dup row r stored at padded row r + DUP
HP2 = HP + DUP               # 114 rows total (2..57 duplicated, shifted left 1)
KS = 3
NTAPS = KS * KS
IKK = C * NTAPS  # 1152
ROWS_PER_TILE = 8           # output rows per PSUM tile -> N = 8*56 = 448
N_TILE = ROWS_PER_TILE * W  # 448 fp32 <= 512 (one PSUM bank)
N_ROW_TILES = H // ROWS_PER_TILE  # 7
ADT = FP8 if USE_FP8 else BF16
ODT = F16 if OUT16 else F32

# synth column chunks: [512, 512, 128] -> three concurrent PSUM groups
# (k-major emission); N >= 256 keeps the big f32r matmuls at full rate
SYN_CHUNKS = ((0, 512), (512, 1024), (1024, IKK))
# x0 arrives in row chunks so sign/conv can start before the full image
X0_CHUNKS = ((0, 16), (16, 36), (36, 56))
XN_CHUNKS = ((0, 28), (28, 56))


def build_program(rv: np.ndarray, n_img: int = B_CORE):
    """Build the per-core Bass program. rv values are baked as immediates."""
    nc = bacc.Bacc(
        "TRN2",
        target_bir_lowering=False,
        debug=False,
        num_devices=N_CORES,
    )

    x_t = nc.dram_tensor("x", (n_img, C, H, W), F32, kind="ExternalInput").ap()
    a_t = nc.dram_tensor("Alpha", (O, 1, 1), F32, kind="ExternalInput").ap()
    IS = C // N_CORES  # in-channels synthesized per core in the CC path
    if CC:
        m_t = nc.dram_tensor(
            "Ms", (O, IS, KS, KS), F32R, kind="ExternalInput"
        ).ap()
        z_t = nc.dram_tensor(
            "Zs", (5, O, IS, KS, KS), F32R, kind="ExternalInput"
        ).ap()
        bw_stage = nc.dram_tensor("bw_stage", (IS, NTAPS * O), FP8)
        bw_all = nc.dram_tensor(
            "bw_all", (C, NTAPS * O), FP8, addr_space="Shared"
        )
    else:
        m_t = nc.dram_tensor(
            "M", (O, C, KS, KS), F32R, kind="ExternalInput"
        ).ap()
        z_t = nc.dram_tensor(
            "Z", (5, O, C, KS, KS), F32R, kind="ExternalInput"
        ).ap()
    out_t = nc.dram_tensor("out", (n_img, O, H, W), ODT, kind="ExternalOutput").ap()

    rv = np.asarray(rv, dtype=np.float32).reshape(-1)
    assert rv.shape[0] == 5

    with tile.TileContext(nc) as tc:
        with (
            tc.tile_pool(name="const", bufs=1) as const_pool,
            tc.tile_pool(name="wsyn", bufs=1) as wsyn_pool,
            tc.tile_pool(name="imgs", bufs=1) as img_pool,
            tc.tile_pool(name="xstage", bufs=4) as x_pool,
            tc.tile_pool(name="evac", bufs=14) as ev_pool,
            tc.tile_pool(
                name="cpsum", bufs=4 if USE_FP8 else 3, space="PSUM"
            ) as cpsum_pool,
            tc.tile_pool(name="spsum", bufs=1, space="PSUM") as spsum_pool,
            tc.tile_pool(name="tpsum", bufs=1, space="PSUM") as tpsum_pool,
        ):
            # --- constants first: warm rhs + rv_k identities build on
            # gpsimd while the first DMAs stream.
            warm_rhs = const_pool.tile([128, 448], BF16)
            nc.gpsimd.memset(warm_rhs, 0.0)
            identity = const_pool.tile([128, 128], BF16)
            make_identity(nc, identity)
            rvI_f32s = []
            for k in range(6):
                # k=5 is the plain identity (1.0): stationary for the
                # M-initializing matmul of each synthesis group
                t32 = const_pool.tile([128, 128], F32, name=f"rvI_f32_{k}")
                nc.gpsimd.memset(t32, 0.0)
                nc.gpsimd.affine_select(
                    out=t32,
                    in_=t32,
                    compare_op=mybir.AluOpType.not_equal,
                    fill=float(rv[k]) if k < 5 else 1.0,
                    base=0,
                    pattern=[[-1, 128]],
                    channel_multiplier=1,
                )
                rvI_f32s.append(t32)

            # --- head DMAs on TWO HWDGE queues (sync + scalar): a single
            # queue only sustains ~300 GB/s, two reach the fabric limit.
            # Per-queue FIFO order == priority: Z in synthesis order, then
            # x0 row chunks, then x1..x3.
            if CC:
                m_sb = wsyn_pool.tile([O, IS * NTAPS], F32R)
                zs_sb = wsyn_pool.tile([O, 5 * IS * NTAPS], F32R)
            else:
                m_sb = wsyn_pool.tile([O, IKK], F32R)
                z_sbs = [
                    wsyn_pool.tile([O, IKK], F32R, name=f"z{k}", tag=f"z{k}")
                    for k in range(5)
                ]
            x_tiles = [
                x_pool.tile([C, H * W], F32, name=f"x{img}", tag="xin")
                for img in range(n_img)
            ]

            def xdma(img, r0, r1):
                xv = x_tiles[img].rearrange("c (h w) -> c h w", w=W)
                return (xv[:, r0:r1, :], x_t[img, :, r0:r1, :])

            alpha_sb = const_pool.tile([O, 1], F32)
            nc.scalar.dma_start(alpha_sb, a_t.rearrange("o a b -> o (a b)"))
            # Z streams in chunk-major order (cc outer, k inner) so each
            # synthesis group's operands land back-to-back and the group
            # completes during the stream. Chunks alternate between the
            # sync and vector trigger engines: one queue only reaches
            # ~300 GB/s, two saturate the fabric, and per-queue FIFO still
            # preserves the chunk-major arrival order.
            # x0's first chunk leads: it absorbs the DMA queue's slow ramp
            # (~2us at <200 GB/s) with non-critical bytes.
            nc.sync.dma_start(*xdma(0, *X0_CHUNKS[0]))
            if CC:
                nc.sync.dma_start(
                    m_sb, m_t.rearrange("o i kh kw -> o (i kh kw)")
                )
                nc.sync.dma_start(
                    zs_sb.rearrange("o (k n) -> o k n", k=5),
                    z_t.rearrange("k o i kh kw -> o k (i kh kw)"),
                )
            else:
                nc.sync.dma_start(
                    m_sb, m_t.rearrange("o i kh kw -> o (i kh kw)")
                )
                for k in range(5):
                    nc.sync.dma_start(
                        z_sbs[k], z_t[k].rearrange("o i kh kw -> o (i kh kw)")
                    )
            for r0, r1 in X0_CHUNKS[1:]:
                nc.sync.dma_start(*xdma(0, r0, r1))
            for img in range(1, n_img):
                nc.sync.dma_start(
                    x_tiles[img], x_t[img].rearrange("c h w -> c (h w)")
                )
            rvI = []
            for k in range(6):
                t = const_pool.tile([128, 128], F32R, name=f"rvI{k}")
                nc.scalar.copy(t, rvI_f32s[k])
                rvI.append(t)

            # --- PE warm-up: keep the HAM clock gate ramping while the
            # head DMAs stream, so neither synth nor conv starts cold.
            warm_ps = cpsum_pool.tile([O, N_TILE], F32, tag="cv")
            for _ in range(N_WARM):
                nc.tensor.matmul(
                    warm_ps, identity, warm_rhs, start=True, stop=True
                )

            # --- per-image padded sign(x) buffers (borders zeroed once) ---
            padded = []
            for img in range(n_img):
                pd = img_pool.tile(
                    [C, HP2 * WP], ADT, name=f"pad{img}", tag=f"pad{img}"
                )
                pd3 = pd.rearrange("p (h w) -> p h w", w=WP)
                nc.gpsimd.memset(pd3[:, 0, 0:HP], 0.0)
                nc.gpsimd.memset(pd3[:, HP - 1, 0:HP], 0.0)
                nc.gpsimd.memset(pd3[:, 1 : HP - 1, 0:1], 0.0)
                nc.gpsimd.memset(pd3[:, 1 : HP - 1, HP - 1 : HP], 0.0)
                # dup of bottom-pad row 57 (zero); dup cols >= 56 are unread
                nc.gpsimd.memset(pd3[:, HP2 - 1, 0:HP], 0.0)
                padded.append(pd3)

            # --- weight synthesis on PE: I @ M starts each PSUM group,
            # then 5 f32r matmuls accumulate rv_k Z_k, sign from PSUM.
            # bwg is the unified conv weight tile [C, (tap, O)].
            bwg = wsyn_pool.tile([C, NTAPS * O], ADT)
            bwg3 = bwg.rearrange("p (t o) -> p t o", o=O)
            if CC:
                bw_nat = wsyn_pool.tile([O, IS * NTAPS], BF16)
                bw3 = bw_nat.rearrange("o (i t) -> o i t", t=NTAPS)
                bw_small = wsyn_pool.tile([IS, NTAPS * O], FP8)
                syn_s = spsum_pool.tile([O, IS * NTAPS], F32)
                tpXa = tpsum_pool.tile([IS, 4 * O], BF16)
                tpXb = tpsum_pool.tile([IS, 5 * O], BF16)
            else:
                bw_nat = wsyn_pool.tile([O, IKK], BF16)
                bw3 = bw_nat.rearrange("o (i t) -> o i t", t=NTAPS)
                syn = [
                    spsum_pool.tile([O, c1 - c0], F32, name=f"syn{i}")
                    for i, (c0, c1) in enumerate(SYN_CHUNKS)
                ]
                if USE_FP8:
                    # one 5-tap-wide PSUM tile (1280B = 1 bank) written in
                    # two waves with a pack between: frees a PSUM bank for
                    # the conv pool (cpsum 4) vs. the two-tile layout
                    tpA = tpsum_pool.tile([128, 5 * O], BF16)
                else:
                    tpP = tpsum_pool.tile([128, 4 * O], BF16)
                    tpS = tpsum_pool.tile([128, 5 * O], BF16)

            def transpose_block(ic, t0, t1, base):
                psl = slice(ic * 32, (ic + 1) * 32)
                for t in range(t0, t1):
                    nc.tensor.transpose(
                        tpA[psl, (t - base) * O : (t - base + 1) * O],
                        bw3[:, psl, t],
                        identity,
                        tile_position=(0, ic * 32),
                    )

            def transpose_chunk(ic):
                psl = slice(ic * 32, (ic + 1) * 32)
                for t in range(NTAPS):
                    dst, toff = (tpP, t * O) if t < 4 else (tpS, (t - 4) * O)
                    nc.tensor.transpose(
                        dst[psl, toff : toff + O],
                        bw3[:, psl, t],
                        identity,
                        tile_position=(0, ic * 32),
                    )

            # --- signs: main rows + shifted dup rows, chunked + interleaved
            # on ACT so conv tiles release as x rows land.
            def sign_main_rows(img, r0, r1):
                # pd rows 1+r0 .. 1+r1 <- sign(x rows r0..r1)
                pd3 = padded[img]
                xi = x_tiles[img].rearrange("c (h w) -> c h w", w=W)
                nc.scalar.sign(
                    pd3[:, 1 + r0 : 1 + r1, 1 : 1 + W], xi[:, r0:r1, :]
                )

            def sign_dup_rows(img, r0, r1):
                # dup rows r (=pd row DUP+r), r in [r0,r1) subset of [2,57):
                # dup[r][c] = pd[r][c+1] = sign(x[r-1][c]), c < 56
                pd3 = padded[img]
                xi = x_tiles[img].rearrange("c (h w) -> c h w", w=W)
                nc.scalar.sign(
                    pd3[:, DUP + r0 : DUP + r1, 0:W],
                    xi[:, r0 - 1 : r1 - 1, :],
                )

            def sign_image_chunk(img, r0, r1):
                sign_main_rows(img, r0, r1)
                sign_dup_rows(img, max(2, r0 + 1), min(57, r1 + 1))

            def sign_image(img):
                for r0, r1 in XN_CHUNKS:
                    sign_image_chunk(img, r0, r1)

            if CC:
                # sharded synthesis: this core owns IS in-channels; one
                # PSUM group (I @ Ms + sum rv_k Zs_k), sign, transpose to
                # [IS, (t, o)], pack fp8, AllGather the 8 slices into the
                # full lhsT and load it back.
                sign_image_chunk(0, *X0_CHUNKS[0])
                nc.tensor.matmul(
                    syn_s, rvI[5], m_sb, start=True, stop=False
                )
                NS = IS * NTAPS
                for k in range(5):
                    nc.tensor.matmul(
                        syn_s,
                        rvI[k],
                        zs_sb[:, k * NS : (k + 1) * NS],
                        start=False,
                        stop=(k == 4),
                    )
                nc.scalar.sign(bw_nat, syn_s)
                for t in range(NTAPS):
                    dst, toff = (
                        (tpXa, t * O) if t < 4 else (tpXb, (t - 4) * O)
                    )
                    nc.tensor.transpose(
                        dst[:, toff : toff + O], bw3[:, :, t], identity
                    )
                nc.vector.tensor_copy(bw_small[:, : 4 * O], tpXa)
                nc.vector.tensor_copy(bw_small[:, 4 * O :], tpXb)
                nc.sync.dma_start(bw_stage.ap(), bw_small)
                nc.gpsimd.collective_compute(
                    "AllGather",
                    mybir.AluOpType.bypass,
                    replica_groups=[list(range(N_CORES))],
                    ins=[bw_stage.ap().opt()],
                    outs=[bw_all.ap().opt()],
                )
                nc.sync.dma_start(bwg, bw_all.ap())
                sign_image_chunk(0, *X0_CHUNKS[1])
                sign_image_chunk(0, *X0_CHUNKS[2])
            else:
                # k-major: each whole-Z arrival releases one matmul per
                # chunk group; the three PSUM groups interleave on the PE
                # queue (legal — the earlier corruption here was the
                # engine-write-to-PSUM race, since fixed by entering M as
                # each group's start matmul). After z4 only the last k's
                # three matmuls + signs + transposes remain. Standalone
                # LDWEIGHTS fill the arrival gaps to hold the HAM clock.
                for cc, (c0, c1) in enumerate(SYN_CHUNKS):
                    nc.tensor.matmul(
                        syn[cc], rvI[5], m_sb[:, c0:c1], start=True, stop=False
                    )
                for k in range(5):
                    for cc, (c0, c1) in enumerate(SYN_CHUNKS):
                        nc.tensor.matmul(
                            syn[cc],
                            rvI[k],
                            z_sbs[k][:, c0:c1],
                            start=False,
                            stop=(k == 4),
                        )
                    if k < 4:
                        for _ in range(6):
                            nc.tensor.ldweights(warm_rhs[:, 0:128])
                # signs chase the k4 matmuls; transpose blocks chase the
                # signs that cover their i-range (block b = cols 288b..)
                nc.scalar.sign(bw_nat[:, 0:512], syn[0])
                sign_image_chunk(0, *X0_CHUNKS[0])
                if USE_FP8:
                    transpose_block(0, 0, 5, 0)
                    nc.scalar.sign(bw_nat[:, 512:1024], syn[1])
                    sign_image_chunk(0, *X0_CHUNKS[1])
                    transpose_block(1, 0, 5, 0)
                    transpose_block(2, 0, 5, 0)
                    nc.scalar.sign(bw_nat[:, 1024:IKK], syn[2])
                    sign_image_chunk(0, *X0_CHUNKS[2])
                    transpose_block(3, 0, 5, 0)
                    nc.vector.tensor_copy(bwg[:, : 5 * O], tpA)
                    for ic in range(4):
                        transpose_block(ic, 5, NTAPS, 5)
                    nc.vector.tensor_copy(
                        bwg[:, 5 * O :], tpA[:, : 4 * O]
                    )
                else:
                    transpose_chunk(0)
                    nc.scalar.sign(bw_nat[:, 512:1024], syn[1])
                    sign_image_chunk(0, *X0_CHUNKS[1])
                    transpose_chunk(1)
                    transpose_chunk(2)
                    nc.scalar.sign(bw_nat[:, 1024:IKK], syn[2])
                    sign_image_chunk(0, *X0_CHUNKS[2])
                    transpose_chunk(3)
                    nc.vector.tensor_copy(
                        bwg3[:, 0:4, :],
                        tpP.rearrange("p (t o) -> p t o", o=O),
                    )
                    nc.vector.tensor_copy(
                        bwg3[:, 4:NTAPS, :],
                        tpS.rearrange("p (t o) -> p t o", o=O),
                    )

            # --- main conv loop; next image's sign emitted before this
            # image's tiles so ACT never head-of-line blocks ---
            for img in range(n_img):
                if img + 1 < n_img:
                    sign_image(img + 1)
                pd3 = padded[img]

                for nt in range(N_ROW_TILES):
                    y0 = nt * ROWS_PER_TILE
                    cv = cpsum_pool.tile([O, N_TILE], F32, tag="cv")
                    if USE_FP8:
                        # 3 vertical pairs {(0,kx),(1,kx)}: rhs pair step
                        # WP, lhsT pair = taps (kx, kx+3) at stride 3*O
                        for kx in range(KS):
                            win0 = pd3[:, y0 : y0 + ROWS_PER_TILE, kx : kx + W]
                            ap4 = bass.AP(
                                win0.tensor,
                                win0.offset,
                                [list(win0.ap[0]), [WP, 2]]
                                + [list(p) for p in win0.ap[1:]],
                            )
                            wv = bwg3[:, kx, :]
                            apW = bass.AP(
                                wv.tensor,
                                wv.offset,
                                [list(wv.ap[0]), [3 * O, 2], [1, O]],
                            )
                            nc.tensor.matmul(
                                cv,
                                apW,
                                ap4,
                                start=(kx == 0),
                                stop=False,
                                perf_mode=mybir.MatmulPerfMode.DoubleRow,
                            )
                        # pair {(2,0),(2,1)}: elem 1 in the shifted dup rows
                        winD = pd3[:, y0 + 2 : y0 + 2 + ROWS_PER_TILE, 0:W]
                        apD = bass.AP(
                            winD.tensor,
                            winD.offset,
                            [list(winD.ap[0]), [DUP * WP, 2]]
                            + [list(p) for p in winD.ap[1:]],
                        )
                        nc.tensor.matmul(
                            cv,
                            bwg3[:, 6:8, :],
                            apD,
                            start=False,
                            stop=False,
                            perf_mode=mybir.MatmulPerfMode.DoubleRow,
                        )
                        # single tap (2,2)
                        winS = pd3[
                            :, y0 + 2 : y0 + 2 + ROWS_PER_TILE, 2 : 2 + W
                        ]
                        nc.tensor.matmul(
                            cv, bwg3[:, 8, :], winS, start=False, stop=True
                        )
                    else:
                        t = 0
                        for ky in range(KS):
                            for kx in range(KS):
                                win = pd3[
                                    :,
                                    y0 + ky : y0 + ky + ROWS_PER_TILE,
                                    kx : kx + W,
                                ]
                                nc.tensor.matmul(
                                    cv,
                                    bwg3[:, t, :],
                                    win,
                                    start=(t == 0),
                                    stop=(t == NTAPS - 1),
                                )
                                t += 1
                    ev = ev_pool.tile([O, N_TILE], ODT, tag="ev")
                    nc.vector.tensor_scalar_mul(ev, cv, alpha_sb[:, 0:1])
                    # alternate store queues; keep the final stores on the
                    # low-latency HWDGE path so the tail drains fast
                    tile_n = img * N_ROW_TILES + nt
                    eng = (
                        nc.sync
                        if (tile_n % 2 == 0 or tile_n >= 26)
                        else nc.gpsimd
                    )
                    eng.dma_start(
                        out_t[img, :, y0 : y0 + ROWS_PER_TILE, :],
                        ev.rearrange("o (h w) -> o h w", w=W),
                    )

    nc.compile()
    return nc


def _ensure_ntff_hook():
    """Register the axon NTFF profiling hook if the image's antenv lacks it.

    Only used when BASS_KERNEL_TRACE=1 (dev profiling); best-effort.
    """
    import sys
    import types

    try:
        import antenv

        if hasattr(antenv, "axon_hooks"):
            return
        mod = types.ModuleType("antenv.axon_hooks")
        _hook = [None]
        mod.set_axon_ntff_profile_hook = lambda h: _hook.__setitem__(0, h)
        mod.get_axon_ntff_profile_hook = lambda: _hook[0]
        sys.modules["antenv.axon_hooks"] = mod
        antenv.axon_hooks = mod
        from trn_agent_boot.trn_boot import _ntff_profile_via_ctypes

        mod.set_axon_ntff_profile_hook(
            _ntff_profile_via_ctypes("/opt/axon/libaxon_pjrt.so")
        )
    except Exception as e:  # pragma: no cover - profiling is optional
        print(f"NTFF hook registration failed ({e}); tracing disabled")


def kernel(x, Alpha, M, Z, rv):
    x = np.ascontiguousarray(np.asarray(x, dtype=np.float32))
    Alpha = np.ascontiguousarray(np.asarray(Alpha, dtype=np.float32))
    M = np.ascontiguousarray(np.asarray(M, dtype=np.float32))
    Z = np.ascontiguousarray(np.asarray(Z, dtype=np.float32))
    rv = np.asarray(rv, dtype=np.float32)

    trace = bool(int(os.environ.get("BASS_KERNEL_TRACE", "0")))
    if trace:
        _ensure_ntff_hook()

    nc = build_program(rv)

    IS = C // N_CORES
    in_maps = []
    for c in range(N_CORES):
        im = {
            "x": np.ascontiguousarray(x[c * B_CORE : (c + 1) * B_CORE]),
            "Alpha": Alpha,
        }
        if CC:
            im["Ms"] = np.ascontiguousarray(M[:, c * IS : (c + 1) * IS])
            im["Zs"] = np.ascontiguousarray(Z[:, :, c * IS : (c + 1) * IS])
        else:
            im["M"] = M
            im["Z"] = Z
        in_maps.append(im)

    res = run_bass_kernel_spmd(
        nc,
        in_maps,
        core_ids=list(range(N_CORES)),
        trace=trace,
    )
    out = np.concatenate(
        [res.results[c]["out"] for c in range(N_CORES)], axis=0
    ).astype(np.float32)
    if trace:
        kernel.last_results = res
    return out


# revision 73
# speedup vs baseline: 1.0423x; 1.0423x over previous
"""Trainium2 Bass kernel for BinarizeConv2dSDP.

Math (reference):
    s   = M + rv @ Z          (the rsqrt normalization is sign-preserving:
                               w = (m + rv@z) * rsqrt(...) with rsqrt > 0,
                               so sign(w) == sign(s))
    bw  = sign(s)             (O, I, 3, 3)
    ba  = sign(x)             (B, C, H, W)
    out = conv2d(ba, bw, stride 1, pad 1) * Alpha

Strategy:
    - Data-parallel over batch: 8 cores x 4 images each. M/Z/Alpha replicated.
    - Weight synthesis on the PE: per 288-col chunk, one contiguous PSUM
      accumulation group [I @ M, then rv_k I @ Z_k x5] of fp32r matmuls at
      full rate (N >= 256); ACT signs straight from PSUM, PE transposes the
      signed taps into lhsT layout (lag-one behind the group signs), DVE
      packs to fp8. Only Z sees fp32r input rounding -> a handful of
      borderline sign flips, well under the 2e-2 gate.
      HW rules learned the hard way: accumulation groups must stay
      contiguous on the PE queue (no other matmul interleaved), and an
      engine write to PSUM followed by matmul accumulation races; only
      standalone LDWEIGHTS may fill arrival gaps (HAM clock-gate warm-up,
      along with filler matmuls before the stream).
    - Head DMAs on one sync queue, few big transfers (the trigger rate
      otherwise starves the wire; a second queue halves per-queue rate):
      x0's first row-chunk leads to absorb the queue's slow ramp, then M,
      Z as L(864)/R(288) column halves so chunk groups c0-c2 finish while
      the R half streams, then the rest of x. Image-0 sign chunks slot
      between the weight signs on ACT's FIFO.
    - Binarized conv, 5 passes per 8-row tile (was 6): sign(x) lives in a
      zero-padded [128, 115 x 64] fp8 image; rows 59..114 hold a one-column-
      left-shifted duplicate of rows 2..57. Vertical tap pairs (ky 0+1) use
      DoubleRow with rhs pair step 64 and lhsT pair stride 3*O; the
      (2,0)+(2,1) pair uses DoubleRow into the duplicate (rhs pair step
      57*64); (2,2) is a single matmul. +-1 is exact in fp8/bf16 and PSUM
      accumulates f32 -> integer-exact.
    - Evacuation (x Alpha) on DVE; output stored fp16 (conv integers
      <= 1152 are fp16-exact; only the Alpha product rounds, ~5e-4), host
      upcasts. Stores alternate sync/gpsimd queues, tail on sync.
    - BASS_KERNEL_CC=1 enables a sharded-synthesis variant (each core
      synthesizes 16 in-channels, AllGather rebuilds the lhsT): correct,
      but the collective eats the full SPMD core-launch skew (~50us on
      this stack), so it stays off.
"""

import os
import numpy as np

import concourse.bass as bass
import concourse.tile as tile
from concourse import bacc, mybir
from concourse.bass_utils import run_bass_kernel_spmd
from concourse.masks import make_identity

F32 = mybir.dt.float32
F32R = mybir.dt.float32r
F16 = mybir.dt.float16
BF16 = mybir.dt.bfloat16
FP8 = mybir.dt.float8e4

USE_FP8 = bool(int(os.environ.get("BASS_KERNEL_FP8", "1")))
OUT16 = bool(int(os.environ.get("BASS_KERNEL_OUT16", "1")))
N_WARM = int(os.environ.get("BASS_KERNEL_WARM", "12"))
# sharded weight synthesis + AllGather: correct, but the collective pays
# the full SPMD core-launch skew (~50us on this stack) -> off by default
CC = bool(int(os.environ.get("BASS_KERNEL_CC", "0"))) and USE_FP8

B_FULL = 32
N_CORES = 8
B_CORE = B_FULL // N_CORES  # 4 images per core
C = 128      # in channels
O = 128      # out channels
H = W = 56
HP = 58                      # padded rows
WP = 64 if USE_FP8 else 58   # padded row stride (64 -> vertical pair step 64B)
DUP = 57                     # dup row r stored at padded row r + DUP
HP2 = HP + DUP               # 114 rows total (2..57 duplicated, shifted left 1)
KS = 3
NTAPS = KS * KS
IKK = C * NTAPS  # 1152
ROWS_PER_TILE = 8           # output rows per PSUM tile -> N = 8*56 = 448
N_TILE = ROWS_PER_TILE * W  # 448 fp32 <= 512 (one PSUM bank)
N_ROW_TILES = H // ROWS_PER_TILE  # 7
ADT = FP8 if USE_FP8 else BF16
ODT = F16 if OUT16 else F32

# synth column chunks: [512, 512, 128] -> three concurrent PSUM groups
# (k-major emission); N >= 256 keeps the big f32r matmuls at full rate
SYN_CHUNKS = ((0, 512), (512, 1024), (1024, IKK))
# x0 arrives in row chunks so sign/conv can start before the full image
X0_CHUNKS = ((0, 16), (16, 36), (36, 56))
XN_CHUNKS = ((0, 28), (28, 56))


def build_program(rv: np.ndarray, n_img: int = B_CORE):
    """Build the per-core Bass program. rv values are baked as immediates."""
    nc = bacc.Bacc(
        "TRN2",
        target_bir_lowering=False,
        debug=False,
        num_devices=N_CORES,
    )

    x_t = nc.dram_tensor("x", (n_img, C, H, W), F32, kind="ExternalInput").ap()
    a_t = nc.dram_tensor("Alpha", (O, 1, 1), F32, kind="ExternalInput").ap()
    IS = C // N_CORES  # in-channels synthesized per core in the CC path
    if CC:
        m_t = nc.dram_tensor(
            "Ms", (O, IS, KS, KS), F32R, kind="ExternalInput"
        ).ap()
        z_t = nc.dram_tensor(
            "Zs", (5, O, IS, KS, KS), F32R, kind="ExternalInput"
        ).ap()
        bw_stage = nc.dram_tensor("bw_stage", (IS, NTAPS * O), FP8)
        bw_all = nc.dram_tensor(
            "bw_all", (C, NTAPS * O), FP8, addr_space="Shared"
        )
    else:
        m_t = nc.dram_tensor(
            "M", (O, C, KS, KS), F32R, kind="ExternalInput"
        ).ap()
        z_t = nc.dram_tensor(
            "Z", (5, O, C, KS, KS), F32R, kind="ExternalInput"
        ).ap()
    out_t = nc.dram_tensor("out", (n_img, O, H, W), ODT, kind="ExternalOutput").ap()

    rv = np.asarray(rv, dtype=np.float32).reshape(-1)
    assert rv.shape[0] == 5

    with tile.TileContext(nc) as tc:
        with (
            tc.tile_pool(name="const", bufs=1) as const_pool,
            tc.tile_pool(name="wsyn", bufs=1) as wsyn_pool,
            tc.tile_pool(name="imgs", bufs=1) as img_pool,
            tc.tile_pool(name="xstage", bufs=4) as x_pool,
            tc.tile_pool(name="evac", bufs=14) as ev_pool,
            tc.tile_pool(name="cpsum", bufs=4, space="PSUM") as cpsum_pool,
            tc.tile_pool(name="spsum", bufs=1, space="PSUM") as spsum_pool,
            tc.tile_pool(name="tpsum", bufs=1, space="PSUM") as tpsum_pool,
        ):
            # --- constants first: warm rhs + rv_k identities build on
            # gpsimd while the first DMAs stream.
            warm_rhs = const_pool.tile([128, 448], BF16)
            nc.gpsimd.memset(warm_rhs, 0.0)
            identity = const_pool.tile([128, 128], BF16)
            make_identity(nc, identity)
            rvI_f32s = []
            for k in range(6):
                # k=5 is the plain identity (1.0): stationary for the
                # M-initializing matmul of each synthesis group
                t32 = const_pool.tile([128, 128], F32, name=f"rvI_f32_{k}")
                nc.gpsimd.memset(t32, 0.0)
                nc.gpsimd.affine_select(
                    out=t32,
                    in_=t32,
                    compare_op=mybir.AluOpType.not_equal,
                    fill=float(rv[k]) if k < 5 else 1.0,
                    base=0,
                    pattern=[[-1, 128]],
                    channel_multiplier=1,
                )
                rvI_f32s.append(t32)

            # --- head DMAs on TWO HWDGE queues (sync + scalar): a single
            # queue only sustains ~300 GB/s, two reach the fabric limit.
            # Per-queue FIFO order == priority: Z in synthesis order, then
            # x0 row chunks, then x1..x3.
            if CC:
                m_sb = wsyn_pool.tile([O, IS * NTAPS], F32R)
                zs_sb = wsyn_pool.tile([O, 5 * IS * NTAPS], F32R)
            else:
                m_sb = wsyn_pool.tile([O, IKK], F32R)
                z_sbs = [
                    wsyn_pool.tile([O, IKK], F32R, name=f"z{k}", tag=f"z{k}")
                    for k in range(5)
                ]
            x_tiles = [
                x_pool.tile([C, H * W], F32, name=f"x{img}", tag="xin")
                for img in range(n_img)
            ]

            def xdma(img, r0, r1):
                xv = x_tiles[img].rearrange("c (h w) -> c h w", w=W)
                return (xv[:, r0:r1, :], x_t[img, :, r0:r1, :])

            alpha_sb = const_pool.tile([O, 1], F32)
            nc.scalar.dma_start(alpha_sb, a_t.rearrange("o a b -> o (a b)"))
            # Z streams in chunk-major order (cc outer, k inner) so each
            # synthesis group's operands land back-to-back and the group
            # completes during the stream. Chunks alternate between the
            # sync and vector trigger engines: one queue only reaches
            # ~300 GB/s, two saturate the fabric, and per-queue FIFO still
            # preserves the chunk-major arrival order.
            # x0's first chunk leads: it absorbs the DMA queue's slow ramp
            # (~2us at <200 GB/s) with non-critical bytes.
            nc.sync.dma_start(*xdma(0, *X0_CHUNKS[0]))
            if CC:
                nc.sync.dma_start(
                    m_sb, m_t.rearrange("o i kh kw -> o (i kh kw)")
                )
                nc.sync.dma_start(
                    zs_sb.rearrange("o (k n) -> o k n", k=5),
                    z_t.rearrange("k o i kh kw -> o k (i kh kw)"),
                )
            else:
                nc.sync.dma_start(
                    m_sb, m_t.rearrange("o i kh kw -> o (i kh kw)")
                )
                for k in range(5):
                    nc.sync.dma_start(
                        z_sbs[k], z_t[k].rearrange("o i kh kw -> o (i kh kw)")
                    )
            for r0, r1 in X0_CHUNKS[1:]:
                nc.sync.dma_start(*xdma(0, r0, r1))
            for img in range(1, n_img):
                nc.sync.dma_start(
                    x_tiles[img], x_t[img].rearrange("c h w -> c (h w)")
                )
            rvI = []
            for k in range(6):
                t = const_pool.tile([128, 128], F32R, name=f"rvI{k}")
                nc.scalar.copy(t, rvI_f32s[k])
                rvI.append(t)

            # --- PE warm-up: keep the HAM clock gate ramping while the
            # head DMAs stream, so neither synth nor conv starts cold.
            warm_ps = cpsum_pool.tile([O, N_TILE], F32, tag="cv")
            for _ in range(N_WARM):
                nc.tensor.matmul(
                    warm_ps, identity, warm_rhs, start=True, stop=True
                )

            # --- per-image padded sign(x) buffers (borders zeroed once) ---
            padded = []
            for img in range(n_img):
                pd = img_pool.tile(
                    [C, HP2 * WP], ADT, name=f"pad{img}", tag=f"pad{img}"
                )
                pd3 = pd.rearrange("p (h w) -> p h w", w=WP)
                nc.gpsimd.memset(pd3[:, 0, 0:HP], 0.0)
                nc.gpsimd.memset(pd3[:, HP - 1, 0:HP], 0.0)
                nc.gpsimd.memset(pd3[:, 1 : HP - 1, 0:1], 0.0)
                nc.gpsimd.memset(pd3[:, 1 : HP - 1, HP - 1 : HP], 0.0)
                # dup of bottom-pad row 57 (zero); dup cols >= 56 are unread
                nc.gpsimd.memset(pd3[:, HP2 - 1, 0:HP], 0.0)
                padded.append(pd3)

            # --- weight synthesis on PE: I @ M starts each PSUM group,
            # then 5 f32r matmuls accumulate rv_k Z_k, sign from PSUM.
            # bwg is the unified conv weight tile [C, (tap, O)].
            bwg = wsyn_pool.tile([C, NTAPS * O], ADT)
            bwg3 = bwg.rearrange("p (t o) -> p t o", o=O)
            if CC:
                bw_nat = wsyn_pool.tile([O, IS * NTAPS], BF16)
                bw3 = bw_nat.rearrange("o (i t) -> o i t", t=NTAPS)
                bw_small = wsyn_pool.tile([IS, NTAPS * O], FP8)
                syn_s = spsum_pool.tile([O, IS * NTAPS], F32)
                tpXa = tpsum_pool.tile([IS, 4 * O], BF16)
                tpXb = tpsum_pool.tile([IS, 5 * O], BF16)
            else:
                bw_nat = wsyn_pool.tile([O, IKK], BF16)
                bw3 = bw_nat.rearrange("o (i t) -> o i t", t=NTAPS)
                # the third (128-col) synthesis group borrows the warm-up
                # conv-PSUM tile: its group finishes and is signed long
                # before conv tile 3 recycles that bank, keeping all of
                # cpsum=4 + syn 2 + transpose 2 within the 8 banks
                syn = [
                    spsum_pool.tile([O, 512], F32, name="syn0"),
                    spsum_pool.tile([O, 512], F32, name="syn1"),
                    warm_ps[:, 0:128],
                ]
                tpP = tpsum_pool.tile([128, 4 * O], BF16)
                tpS = tpsum_pool.tile([128, 5 * O], BF16)

            def transpose_chunk(ic):
                psl = slice(ic * 32, (ic + 1) * 32)
                for t in range(NTAPS):
                    dst, toff = (tpP, t * O) if t < 4 else (tpS, (t - 4) * O)
                    nc.tensor.transpose(
                        dst[psl, toff : toff + O],
                        bw3[:, psl, t],
                        identity,
                        tile_position=(0, ic * 32),
                    )

            # --- signs: main rows + shifted dup rows, chunked + interleaved
            # on ACT so conv tiles release as x rows land.
            def sign_main_rows(img, r0, r1):
                # pd rows 1+r0 .. 1+r1 <- sign(x rows r0..r1)
                pd3 = padded[img]
                xi = x_tiles[img].rearrange("c (h w) -> c h w", w=W)
                nc.scalar.sign(
                    pd3[:, 1 + r0 : 1 + r1, 1 : 1 + W], xi[:, r0:r1, :]
                )

            def sign_dup_rows(img, r0, r1):
                # dup rows r (=pd row DUP+r), r in [r0,r1) subset of [2,57):
                # dup[r][c] = pd[r][c+1] = sign(x[r-1][c]), c < 56
                pd3 = padded[img]
                xi = x_tiles[img].rearrange("c (h w) -> c h w", w=W)
                nc.scalar.sign(
                    pd3[:, DUP + r0 : DUP + r1, 0:W],
                    xi[:, r0 - 1 : r1 - 1, :],
                )

            def sign_image_chunk(img, r0, r1):
                sign_main_rows(img, r0, r1)
                sign_dup_rows(img, max(2, r0 + 1), min(57, r1 + 1))

            def sign_image(img):
                for r0, r1 in XN_CHUNKS:
                    sign_image_chunk(img, r0, r1)

            if CC:
                # sharded synthesis: this core owns IS in-channels; one
                # PSUM group (I @ Ms + sum rv_k Zs_k), sign, transpose to
                # [IS, (t, o)], pack fp8, AllGather the 8 slices into the
                # full lhsT and load it back.
                sign_image_chunk(0, *X0_CHUNKS[0])
                nc.tensor.matmul(
                    syn_s, rvI[5], m_sb, start=True, stop=False
                )
                NS = IS * NTAPS
                for k in range(5):
                    nc.tensor.matmul(
                        syn_s,
                        rvI[k],
                        zs_sb[:, k * NS : (k + 1) * NS],
                        start=False,
                        stop=(k == 4),
                    )
                nc.scalar.sign(bw_nat, syn_s)
                for t in range(NTAPS):
                    dst, toff = (
                        (tpXa, t * O) if t < 4 else (tpXb, (t - 4) * O)
                    )
                    nc.tensor.transpose(
                        dst[:, toff : toff + O], bw3[:, :, t], identity
                    )
                nc.vector.tensor_copy(bw_small[:, : 4 * O], tpXa)
                nc.vector.tensor_copy(bw_small[:, 4 * O :], tpXb)
                nc.sync.dma_start(bw_stage.ap(), bw_small)
                nc.gpsimd.collective_compute(
                    "AllGather",
                    mybir.AluOpType.bypass,
                    replica_groups=[list(range(N_CORES))],
                    ins=[bw_stage.ap().opt()],
                    outs=[bw_all.ap().opt()],
                )
                nc.sync.dma_start(bwg, bw_all.ap())
                sign_image_chunk(0, *X0_CHUNKS[1])
                sign_image_chunk(0, *X0_CHUNKS[2])
            else:
                # k-major: each whole-Z arrival releases one matmul per
                # chunk group; the three PSUM groups interleave on the PE
                # queue (legal — the earlier corruption here was the
                # engine-write-to-PSUM race, since fixed by entering M as
                # each group's start matmul). After z4 only the last k's
                # three matmuls + signs + transposes remain. Standalone
                # LDWEIGHTS fill the arrival gaps to hold the HAM clock.
                for cc, (c0, c1) in enumerate(SYN_CHUNKS):
                    nc.tensor.matmul(
                        syn[cc], rvI[5], m_sb[:, c0:c1], start=True, stop=False
                    )
                for k in range(5):
                    for cc, (c0, c1) in enumerate(SYN_CHUNKS):
                        nc.tensor.matmul(
                            syn[cc],
                            rvI[k],
                            z_sbs[k][:, c0:c1],
                            start=False,
                            stop=(k == 4),
                        )
                    if k < 4:
                        for _ in range(6):
                            nc.tensor.ldweights(warm_rhs[:, 0:128])
                # signs chase the k4 matmuls; transpose blocks chase the
                # signs that cover their i-range (block b = cols 288b..)
                nc.scalar.sign(bw_nat[:, 0:512], syn[0])
                sign_image_chunk(0, *X0_CHUNKS[0])
                transpose_chunk(0)
                nc.scalar.sign(bw_nat[:, 512:1024], syn[1])
                sign_image_chunk(0, *X0_CHUNKS[1])
                transpose_chunk(1)
                transpose_chunk(2)
                nc.scalar.sign(bw_nat[:, 1024:IKK], syn[2])
                sign_image_chunk(0, *X0_CHUNKS[2])
                transpose_chunk(3)
                nc.vector.tensor_copy(
                    bwg3[:, 0:4, :],
                    tpP.rearrange("p (t o) -> p t o", o=O),
                )
                nc.vector.tensor_copy(
                    bwg3[:, 4:NTAPS, :],
                    tpS.rearrange("p (t o) -> p t o", o=O),
                )

            # --- main conv loop; next image's sign emitted before this
            # image's tiles so ACT never head-of-line blocks ---
            for img in range(n_img):
                if img + 1 < n_img:
                    sign_image(img + 1)
                pd3 = padded[img]

                for nt in range(N_ROW_TILES):
                    y0 = nt * ROWS_PER_TILE
                    cv = cpsum_pool.tile([O, N_TILE], F32, tag="cv")
                    if USE_FP8:
                        # 3 vertical pairs {(0,kx),(1,kx)}: rhs pair step
                        # WP, lhsT pair = taps (kx, kx+3) at stride 3*O
                        for kx in range(KS):
                            win0 = pd3[:, y0 : y0 + ROWS_PER_TILE, kx : kx + W]
                            ap4 = bass.AP(
                                win0.tensor,
                                win0.offset,
                                [list(win0.ap[0]), [WP, 2]]
                                + [list(p) for p in win0.ap[1:]],
                            )
                            wv = bwg3[:, kx, :]
                            apW = bass.AP(
                                wv.tensor,
                                wv.offset,
                                [list(wv.ap[0]), [3 * O, 2], [1, O]],
                            )
                            nc.tensor.matmul(
                                cv,
                                apW,
                                ap4,
                                start=(kx == 0),
                                stop=False,
                                perf_mode=mybir.MatmulPerfMode.DoubleRow,
                            )
                        # pair {(2,0),(2,1)}: elem 1 in the shifted dup rows
                        winD = pd3[:, y0 + 2 : y0 + 2 + ROWS_PER_TILE, 0:W]
                        apD = bass.AP(
                            winD.tensor,
                            winD.offset,
                            [list(winD.ap[0]), [DUP * WP, 2]]
                            + [list(p) for p in winD.ap[1:]],
                        )
                        nc.tensor.matmul(
                            cv,
                            bwg3[:, 6:8, :],
                            apD,
                            start=False,
                            stop=False,
                            perf_mode=mybir.MatmulPerfMode.DoubleRow,
                        )
                        # single tap (2,2)
                        winS = pd3[
                            :, y0 + 2 : y0 + 2 + ROWS_PER_TILE, 2 : 2 + W
                        ]
                        nc.tensor.matmul(
                            cv, bwg3[:, 8, :], winS, start=False, stop=True
                        )
                    else:
                        t = 0
                        for ky in range(KS):
                            for kx in range(KS):
                                win = pd3[
                                    :,
                                    y0 + ky : y0 + ky + ROWS_PER_TILE,
                                    kx : kx + W,
                                ]
                                nc.tensor.matmul(
                                    cv,
                                    bwg3[:, t, :],
                                    win,
                                    start=(t == 0),
                                    stop=(t == NTAPS - 1),
                                )
                                t += 1
                    ev = ev_pool.tile([O, N_TILE], ODT, tag="ev")
                    nc.vector.tensor_scalar_mul(ev, cv, alpha_sb[:, 0:1])
                    # alternate store queues; keep the final stores on the
                    # low-latency HWDGE path so the tail drains fast
                    tile_n = img * N_ROW_TILES + nt
                    eng = (
                        nc.sync
                        if (tile_n % 2 == 0 or tile_n >= 26)
                        else nc.gpsimd
                    )
                    eng.dma_start(
                        out_t[img, :, y0 : y0 + ROWS_PER_TILE, :],
                        ev.rearrange("o (h w) -> o h w", w=W),
                    )

    nc.compile()
    return nc


def _ensure_ntff_hook():
    """Register the axon NTFF profiling hook if the image's antenv lacks it.

    Only used when BASS_KERNEL_TRACE=1 (dev profiling); best-effort.
    """
    import sys
    import types

    try:
        import antenv

        if hasattr(antenv, "axon_hooks"):
            return
        mod = types.ModuleType("antenv.axon_hooks")
        _hook = [None]
        mod.set_axon_ntff_profile_hook = lambda h: _hook.__setitem__(0, h)
        mod.get_axon_ntff_profile_hook = lambda: _hook[0]
        sys.modules["antenv.axon_hooks"] = mod
        antenv.axon_hooks = mod
        from trn_agent_boot.trn_boot import _ntff_profile_via_ctypes

        mod.set_axon_ntff_profile_hook(
            _ntff_profile_via_ctypes("/opt/axon/libaxon_pjrt.so")
        )
    except Exception as e:  # pragma: no cover - profiling is optional
        print(f"NTFF hook registration failed ({e}); tracing disabled")


def kernel(x, Alpha, M, Z, rv):
    x = np.ascontiguousarray(np.asarray(x, dtype=np.float32))
    Alpha = np.ascontiguousarray(np.asarray(Alpha, dtype=np.float32))
    M = np.ascontiguousarray(np.asarray(M, dtype=np.float32))
    Z = np.ascontiguousarray(np.asarray(Z, dtype=np.float32))
    rv = np.asarray(rv, dtype=np.float32)

    trace = bool(int(os.environ.get("BASS_KERNEL_TRACE", "0")))
    if trace:
        _ensure_ntff_hook()

    nc = build_program(rv)

    IS = C // N_CORES
    in_maps = []
    for c in range(N_CORES):
        im = {
            "x": np.ascontiguousarray(x[c * B_CORE : (c + 1) * B_CORE]),
            "Alpha": Alpha,
        }
        if CC:
            im["Ms"] = np.ascontiguousarray(M[:, c * IS : (c + 1) * IS])
            im["Zs"] = np.ascontiguousarray(Z[:, :, c * IS : (c + 1) * IS])
        else:
            im["M"] = M
            im["Z"] = Z
        in_maps.append(im)

    res = run_bass_kernel_spmd(
        nc,
        in_maps,
        core_ids=list(range(N_CORES)),
        trace=trace,
    )
    out = np.concatenate(
        [res.results[c]["out"] for c in range(N_CORES)], axis=0
    ).astype(np.float32)
    if trace:
        kernel.last_results = res
    return out


# revision 74
# speedup vs baseline: 1.1817x; 1.1337x over previous
"""Trainium2 Bass kernel for BinarizeConv2dSDP.

Math (reference):
    s   = M + rv @ Z          (the rsqrt normalization is sign-preserving:
                               w = (m + rv@z) * rsqrt(...) with rsqrt > 0,
                               so sign(w) == sign(s))
    bw  = sign(s)             (O, I, 3, 3)
    ba  = sign(x)             (B, C, H, W)
    out = conv2d(ba, bw, stride 1, pad 1) * Alpha

Strategy:
    - Data-parallel over batch: 8 cores x 4 images each. M/Z/Alpha replicated.
    - Weight synthesis on the PE: per 288-col chunk, one contiguous PSUM
      accumulation group [I @ M, then rv_k I @ Z_k x5] of fp32r matmuls at
      full rate (N >= 256); ACT signs straight from PSUM, PE transposes the
      signed taps into lhsT layout (lag-one behind the group signs), DVE
      packs to fp8. Only Z sees fp32r input rounding -> a handful of
      borderline sign flips, well under the 2e-2 gate.
      HW rules learned the hard way: accumulation groups must stay
      contiguous on the PE queue (no other matmul interleaved), and an
      engine write to PSUM followed by matmul accumulation races; only
      standalone LDWEIGHTS may fill arrival gaps (HAM clock-gate warm-up,
      along with filler matmuls before the stream).
    - Head DMAs on one sync queue, few big transfers (the trigger rate
      otherwise starves the wire; a second queue halves per-queue rate):
      x0's first row-chunk leads to absorb the queue's slow ramp, then M,
      Z as L(864)/R(288) column halves so chunk groups c0-c2 finish while
      the R half streams, then the rest of x. Image-0 sign chunks slot
      between the weight signs on ACT's FIFO.
    - Binarized conv, 5 passes per 8-row tile (was 6): sign(x) lives in a
      zero-padded [128, 115 x 64] fp8 image; rows 59..114 hold a one-column-
      left-shifted duplicate of rows 2..57. Vertical tap pairs (ky 0+1) use
      DoubleRow with rhs pair step 64 and lhsT pair stride 3*O; the
      (2,0)+(2,1) pair uses DoubleRow into the duplicate (rhs pair step
      57*64); (2,2) is a single matmul. +-1 is exact in fp8/bf16 and PSUM
      accumulates f32 -> integer-exact.
    - Evacuation (x Alpha) on DVE; output stored fp16 (conv integers
      <= 1152 are fp16-exact; only the Alpha product rounds, ~5e-4), host
      upcasts. Stores alternate sync/gpsimd queues, tail on sync.
    - BASS_KERNEL_CC=1 enables a sharded-synthesis variant (each core
      synthesizes 16 in-channels, AllGather rebuilds the lhsT): correct,
      but the collective eats the full SPMD core-launch skew (~50us on
      this stack), so it stays off.
"""

import os
import numpy as np

import concourse.bass as bass
import concourse.tile as tile
from concourse import bacc, mybir
from concourse.bass_utils import run_bass_kernel_spmd
from concourse.masks import make_identity

F32 = mybir.dt.float32
F32R = mybir.dt.float32r
F16 = mybir.dt.float16
BF16 = mybir.dt.bfloat16
FP8 = mybir.dt.float8e4

USE_FP8 = bool(int(os.environ.get("BASS_KERNEL_FP8", "1")))
OUT16 = bool(int(os.environ.get("BASS_KERNEL_OUT16", "1")))
N_WARM = int(os.environ.get("BASS_KERNEL_WARM", "12"))
# sharded weight synthesis + AllGather: correct, but the collective pays
# the full SPMD core-launch skew (~50us on this stack) -> off by default
CC = bool(int(os.environ.get("BASS_KERNEL_CC", "0"))) and USE_FP8

B_FULL = 32
N_CORES = 8
B_CORE = B_FULL // N_CORES  # 4 images per core
C = 128      # in channels
O = 128      # out channels
H = W = 56
HP = 58                      # padded rows
WP = 64 if USE_FP8 else 58   # padded row stride (64 -> vertical pair step 64B)
DUP = 57                     # dup row r stored at padded row r + DUP
HP2 = HP + DUP               # 114 rows total (2..57 duplicated, shifted left 1)
KS = 3
NTAPS = KS * KS
IKK = C * NTAPS  # 1152
ROWS_PER_TILE = 8           # output rows per PSUM tile -> N = 8*56 = 448
N_TILE = ROWS_PER_TILE * W  # 448 fp32 <= 512 (one PSUM bank)
N_ROW_TILES = H // ROWS_PER_TILE  # 7
ADT = FP8 if USE_FP8 else BF16
ODT = F16 if OUT16 else F32

# synth column chunks: 4 x 288, rotating over two PSUM banks; N >= 256
# keeps the f32r matmuls at full rate
SYN_CHUNKS = tuple((c, c + 288) for c in range(0, IKK, 288))
# x0 arrives in row chunks so sign/conv can start before the full image
X0_CHUNKS = ((0, 16), (16, 36), (36, 56))
XN_CHUNKS = ((0, 28), (28, 56))


def build_program(rv: np.ndarray, n_img: int = B_CORE):
    """Build the per-core Bass program. rv values are baked as immediates."""
    nc = bacc.Bacc(
        "TRN2",
        target_bir_lowering=False,
        debug=False,
        num_devices=N_CORES,
    )

    x_t = nc.dram_tensor("x", (n_img, C, H, W), F32, kind="ExternalInput").ap()
    a_t = nc.dram_tensor("Alpha", (O, 1, 1), F32, kind="ExternalInput").ap()
    IS = C // N_CORES  # in-channels synthesized per core in the CC path
    if CC:
        m_t = nc.dram_tensor(
            "Ms", (O, IS, KS, KS), F32R, kind="ExternalInput"
        ).ap()
        z_t = nc.dram_tensor(
            "Zs", (5, O, IS, KS, KS), F32R, kind="ExternalInput"
        ).ap()
        bw_stage = nc.dram_tensor("bw_stage", (IS, NTAPS * O), FP8)
        bw_all = nc.dram_tensor(
            "bw_all", (C, NTAPS * O), FP8, addr_space="Shared"
        )
    else:
        m_t = nc.dram_tensor(
            "M", (O, C, KS, KS), F32R, kind="ExternalInput"
        ).ap()
        z_t = nc.dram_tensor(
            "Z", (5, O, C, KS, KS), F32R, kind="ExternalInput"
        ).ap()
    out_t = nc.dram_tensor("out", (n_img, O, H, W), ODT, kind="ExternalOutput").ap()

    rv = np.asarray(rv, dtype=np.float32).reshape(-1)
    assert rv.shape[0] == 5

    with tile.TileContext(nc) as tc:
        with (
            tc.tile_pool(name="const", bufs=1) as const_pool,
            tc.tile_pool(name="wsyn", bufs=1) as wsyn_pool,
            tc.tile_pool(name="imgs", bufs=1) as img_pool,
            tc.tile_pool(name="xstage", bufs=4) as x_pool,
            tc.tile_pool(name="evac", bufs=14) as ev_pool,
            tc.tile_pool(name="cpsum", bufs=4, space="PSUM") as cpsum_pool,
            tc.tile_pool(name="spsum", bufs=1, space="PSUM") as spsum_pool,
            tc.tile_pool(name="tpsum", bufs=1, space="PSUM") as tpsum_pool,
        ):
            # --- constants first: warm rhs + rv_k identities build on
            # gpsimd while the first DMAs stream.
            warm_rhs = const_pool.tile([128, 448], BF16)
            nc.gpsimd.memset(warm_rhs, 0.0)
            identity = const_pool.tile([128, 128], BF16)
            make_identity(nc, identity)
            rvI_f32s = []
            for k in range(6):
                # k=5 is the plain identity (1.0): stationary for the
                # M-initializing matmul of each synthesis group
                t32 = const_pool.tile([128, 128], F32, name=f"rvI_f32_{k}")
                nc.gpsimd.memset(t32, 0.0)
                nc.gpsimd.affine_select(
                    out=t32,
                    in_=t32,
                    compare_op=mybir.AluOpType.not_equal,
                    fill=float(rv[k]) if k < 5 else 1.0,
                    base=0,
                    pattern=[[-1, 128]],
                    channel_multiplier=1,
                )
                rvI_f32s.append(t32)

            # --- head DMAs on TWO HWDGE queues (sync + scalar): a single
            # queue only sustains ~300 GB/s, two reach the fabric limit.
            # Per-queue FIFO order == priority: Z in synthesis order, then
            # x0 row chunks, then x1..x3.
            if CC:
                m_sb = wsyn_pool.tile([O, IS * NTAPS], F32R)
                zs_sb = wsyn_pool.tile([O, 5 * IS * NTAPS], F32R)
            else:
                m_sb = wsyn_pool.tile([O, IKK], F32R)
                z_sbs = [
                    wsyn_pool.tile([O, IKK], F32R, name=f"z{k}", tag=f"z{k}")
                    for k in range(5)
                ]
            x_tiles = [
                x_pool.tile([C, H * W], F32, name=f"x{img}", tag="xin")
                for img in range(n_img)
            ]

            def xdma(img, r0, r1):
                xv = x_tiles[img].rearrange("c (h w) -> c h w", w=W)
                return (xv[:, r0:r1, :], x_t[img, :, r0:r1, :])

            alpha_sb = const_pool.tile([O, 1], F32)
            nc.scalar.dma_start(alpha_sb, a_t.rearrange("o a b -> o (a b)"))
            # Z streams in chunk-major order (cc outer, k inner) so each
            # synthesis group's operands land back-to-back and the group
            # completes during the stream. Chunks alternate between the
            # sync and vector trigger engines: one queue only reaches
            # ~300 GB/s, two saturate the fabric, and per-queue FIFO still
            # preserves the chunk-major arrival order.
            # x0's first chunk leads: it absorbs the DMA queue's slow ramp
            # (~2us at <200 GB/s) with non-critical bytes.
            nc.sync.dma_start(*xdma(0, *X0_CHUNKS[0]))
            if CC:
                nc.sync.dma_start(
                    m_sb, m_t.rearrange("o i kh kw -> o (i kh kw)")
                )
                nc.sync.dma_start(
                    zs_sb.rearrange("o (k n) -> o k n", k=5),
                    z_t.rearrange("k o i kh kw -> o k (i kh kw)"),
                )
            else:
                nc.sync.dma_start(
                    m_sb, m_t.rearrange("o i kh kw -> o (i kh kw)")
                )
                ZL = slice(0, 3 * 288)
                ZR = slice(3 * 288, IKK)
                for half in (ZL, ZR):
                    for k in range(5):
                        nc.sync.dma_start(
                            z_sbs[k][:, half],
                            z_t[k].rearrange("o i kh kw -> o (i kh kw)")[
                                :, half
                            ],
                        )
            for r0, r1 in X0_CHUNKS[1:]:
                nc.sync.dma_start(*xdma(0, r0, r1))
            for img in range(1, n_img):
                nc.sync.dma_start(
                    x_tiles[img], x_t[img].rearrange("c h w -> c (h w)")
                )
            rvI = []
            for k in range(6):
                t = const_pool.tile([128, 128], F32R, name=f"rvI{k}")
                nc.scalar.copy(t, rvI_f32s[k])
                rvI.append(t)

            # --- PE warm-up: keep the HAM clock gate ramping while the
            # head DMAs stream, so neither synth nor conv starts cold.
            warm_ps = cpsum_pool.tile([O, N_TILE], F32, tag="cv")
            for _ in range(N_WARM):
                nc.tensor.matmul(
                    warm_ps, identity, warm_rhs, start=True, stop=True
                )

            # --- per-image padded sign(x) buffers (borders zeroed once) ---
            padded = []
            for img in range(n_img):
                pd = img_pool.tile(
                    [C, HP2 * WP], ADT, name=f"pad{img}", tag=f"pad{img}"
                )
                pd3 = pd.rearrange("p (h w) -> p h w", w=WP)
                nc.gpsimd.memset(pd3[:, 0, 0:HP], 0.0)
                nc.gpsimd.memset(pd3[:, HP - 1, 0:HP], 0.0)
                nc.gpsimd.memset(pd3[:, 1 : HP - 1, 0:1], 0.0)
                nc.gpsimd.memset(pd3[:, 1 : HP - 1, HP - 1 : HP], 0.0)
                # dup of bottom-pad row 57 (zero); dup cols >= 56 are unread
                nc.gpsimd.memset(pd3[:, HP2 - 1, 0:HP], 0.0)
                padded.append(pd3)

            # --- weight synthesis on PE: I @ M starts each PSUM group,
            # then 5 f32r matmuls accumulate rv_k Z_k, sign from PSUM.
            # bwg is the unified conv weight tile [C, (tap, O)].
            bwg = wsyn_pool.tile([C, NTAPS * O], ADT)
            bwg3 = bwg.rearrange("p (t o) -> p t o", o=O)
            if CC:
                bw_nat = wsyn_pool.tile([O, IS * NTAPS], BF16)
                bw3 = bw_nat.rearrange("o (i t) -> o i t", t=NTAPS)
                bw_small = wsyn_pool.tile([IS, NTAPS * O], FP8)
                syn_s = spsum_pool.tile([O, IS * NTAPS], F32)
                tpXa = tpsum_pool.tile([IS, 4 * O], BF16)
                tpXb = tpsum_pool.tile([IS, 5 * O], BF16)
            else:
                bw_nat = wsyn_pool.tile([O, IKK], BF16)
                bw3 = bw_nat.rearrange("o (i t) -> o i t", t=NTAPS)
                syn = [
                    spsum_pool.tile([O, 288], F32, name=f"syn{i}")
                    for i in range(2)
                ]
                tpP = tpsum_pool.tile([128, 4 * O], BF16)
                tpS = tpsum_pool.tile([128, 5 * O], BF16)

            def transpose_chunk(ic):
                psl = slice(ic * 32, (ic + 1) * 32)
                for t in range(NTAPS):
                    dst, toff = (tpP, t * O) if t < 4 else (tpS, (t - 4) * O)
                    nc.tensor.transpose(
                        dst[psl, toff : toff + O],
                        bw3[:, psl, t],
                        identity,
                        tile_position=(0, ic * 32),
                    )

            # --- signs: main rows + shifted dup rows, chunked + interleaved
            # on ACT so conv tiles release as x rows land.
            def sign_main_rows(img, r0, r1):
                # pd rows 1+r0 .. 1+r1 <- sign(x rows r0..r1)
                pd3 = padded[img]
                xi = x_tiles[img].rearrange("c (h w) -> c h w", w=W)
                nc.scalar.sign(
                    pd3[:, 1 + r0 : 1 + r1, 1 : 1 + W], xi[:, r0:r1, :]
                )

            def sign_dup_rows(img, r0, r1):
                # dup rows r (=pd row DUP+r), r in [r0,r1) subset of [2,57):
                # dup[r][c] = pd[r][c+1] = sign(x[r-1][c]), c < 56
                pd3 = padded[img]
                xi = x_tiles[img].rearrange("c (h w) -> c h w", w=W)
                nc.scalar.sign(
                    pd3[:, DUP + r0 : DUP + r1, 0:W],
                    xi[:, r0 - 1 : r1 - 1, :],
                )

            def sign_image_chunk(img, r0, r1):
                sign_main_rows(img, r0, r1)
                sign_dup_rows(img, max(2, r0 + 1), min(57, r1 + 1))

            def sign_image(img):
                for r0, r1 in XN_CHUNKS:
                    sign_image_chunk(img, r0, r1)

            if CC:
                # sharded synthesis: this core owns IS in-channels; one
                # PSUM group (I @ Ms + sum rv_k Zs_k), sign, transpose to
                # [IS, (t, o)], pack fp8, AllGather the 8 slices into the
                # full lhsT and load it back.
                sign_image_chunk(0, *X0_CHUNKS[0])
                nc.tensor.matmul(
                    syn_s, rvI[5], m_sb, start=True, stop=False
                )
                NS = IS * NTAPS
                for k in range(5):
                    nc.tensor.matmul(
                        syn_s,
                        rvI[k],
                        zs_sb[:, k * NS : (k + 1) * NS],
                        start=False,
                        stop=(k == 4),
                    )
                nc.scalar.sign(bw_nat, syn_s)
                for t in range(NTAPS):
                    dst, toff = (
                        (tpXa, t * O) if t < 4 else (tpXb, (t - 4) * O)
                    )
                    nc.tensor.transpose(
                        dst[:, toff : toff + O], bw3[:, :, t], identity
                    )
                nc.vector.tensor_copy(bw_small[:, : 4 * O], tpXa)
                nc.vector.tensor_copy(bw_small[:, 4 * O :], tpXb)
                nc.sync.dma_start(bw_stage.ap(), bw_small)
                nc.gpsimd.collective_compute(
                    "AllGather",
                    mybir.AluOpType.bypass,
                    replica_groups=[list(range(N_CORES))],
                    ins=[bw_stage.ap().opt()],
                    outs=[bw_all.ap().opt()],
                )
                nc.sync.dma_start(bwg, bw_all.ap())
                sign_image_chunk(0, *X0_CHUNKS[1])
                sign_image_chunk(0, *X0_CHUNKS[2])
            else:
                # c-major: each chunk's accumulation group stays contiguous
                # on the PE queue (interleaving open matmul groups corrupts
                # on HW). The M term enters as the group's start matmul
                # (I @ M) — an engine write into PSUM followed by matmul
                # accumulation races on HW. Standalone LDWEIGHTS between
                # the arrival-paced matmuls of chunk 0 keep the HAM
                # activity window busy; chunk ic's transposes are emitted
                # after group ic+1 (lag-one) so their sign has retired.
                for cc, (c0, c1) in enumerate(SYN_CHUNKS):
                    ps = syn[cc % 2]
                    nc.tensor.matmul(
                        ps, rvI[5], m_sb[:, c0:c1], start=True, stop=False
                    )
                    for k in range(5):
                        nc.tensor.matmul(
                            ps,
                            rvI[k],
                            z_sbs[k][:, c0:c1],
                            start=False,
                            stop=(k == 4),
                        )
                        if cc == 0 and k < 4:
                            for _ in range(5):
                                nc.tensor.ldweights(warm_rhs[:, 0:128])
                    nc.scalar.sign(bw_nat[:, c0:c1], ps)
                    if cc >= 1:
                        transpose_chunk(cc - 1)
                        sign_image_chunk(0, *X0_CHUNKS[cc - 1])
                transpose_chunk(3)
                nc.vector.tensor_copy(
                    bwg3[:, 0:4, :], tpP.rearrange("p (t o) -> p t o", o=O)
                )
                nc.vector.tensor_copy(
                    bwg3[:, 4:NTAPS, :],
                    tpS.rearrange("p (t o) -> p t o", o=O),
                )

            # --- main conv loop; next image's sign emitted before this
            # image's tiles so ACT never head-of-line blocks ---
            for img in range(n_img):
                if img + 1 < n_img:
                    sign_image(img + 1)
                pd3 = padded[img]

                for nt in range(N_ROW_TILES):
                    y0 = nt * ROWS_PER_TILE
                    cv = cpsum_pool.tile([O, N_TILE], F32, tag="cv")
                    if USE_FP8:
                        # 3 vertical pairs {(0,kx),(1,kx)}: rhs pair step
                        # WP, lhsT pair = taps (kx, kx+3) at stride 3*O
                        for kx in range(KS):
                            win0 = pd3[:, y0 : y0 + ROWS_PER_TILE, kx : kx + W]
                            ap4 = bass.AP(
                                win0.tensor,
                                win0.offset,
                                [list(win0.ap[0]), [WP, 2]]
                                + [list(p) for p in win0.ap[1:]],
                            )
                            wv = bwg3[:, kx, :]
                            apW = bass.AP(
                                wv.tensor,
                                wv.offset,
                                [list(wv.ap[0]), [3 * O, 2], [1, O]],
                            )
                            nc.tensor.matmul(
                                cv,
                                apW,
                                ap4,
                                start=(kx == 0),
                                stop=False,
                                perf_mode=mybir.MatmulPerfMode.DoubleRow,
                            )
                        # pair {(2,0),(2,1)}: elem 1 in the shifted dup rows
                        winD = pd3[:, y0 + 2 : y0 + 2 + ROWS_PER_TILE, 0:W]
                        apD = bass.AP(
                            winD.tensor,
                            winD.offset,
                            [list(winD.ap[0]), [DUP * WP, 2]]
                            + [list(p) for p in winD.ap[1:]],
                        )
                        nc.tensor.matmul(
                            cv,
                            bwg3[:, 6:8, :],
                            apD,
                            start=False,
                            stop=False,
                            perf_mode=mybir.MatmulPerfMode.DoubleRow,
                        )
                        # single tap (2,2)
                        winS = pd3[
                            :, y0 + 2 : y0 + 2 + ROWS_PER_TILE, 2 : 2 + W
                        ]
                        nc.tensor.matmul(
                            cv, bwg3[:, 8, :], winS, start=False, stop=True
                        )
                    else:
                        t = 0
                        for ky in range(KS):
                            for kx in range(KS):
                                win = pd3[
                                    :,
                                    y0 + ky : y0 + ky + ROWS_PER_TILE,
                                    kx : kx + W,
                                ]
                                nc.tensor.matmul(
                                    cv,
                                    bwg3[:, t, :],
                                    win,
                                    start=(t == 0),
                                    stop=(t == NTAPS - 1),
                                )
                                t += 1
                    ev = ev_pool.tile([O, N_TILE], ODT, tag="ev")
                    nc.vector.tensor_scalar_mul(ev, cv, alpha_sb[:, 0:1])
                    # alternate store queues; keep the final stores on the
                    # low-latency HWDGE path so the tail drains fast
                    tile_n = img * N_ROW_TILES + nt
                    eng = (
                        nc.sync
                        if (tile_n % 2 == 0 or tile_n >= 26)
                        else nc.gpsimd
                    )
                    eng.dma_start(
                        out_t[img, :, y0 : y0 + ROWS_PER_TILE, :],
                        ev.rearrange("o (h w) -> o h w", w=W),
                    )

    nc.compile()
    return nc


def _ensure_ntff_hook():
    """Register the axon NTFF profiling hook if the image's antenv lacks it.

    Only used when BASS_KERNEL_TRACE=1 (dev profiling); best-effort.
    """
    import sys
    import types

    try:
        import antenv

        if hasattr(antenv, "axon_hooks"):
            return
        mod = types.ModuleType("antenv.axon_hooks")
        _hook = [None]
        mod.set_axon_ntff_profile_hook = lambda h: _hook.__setitem__(0, h)
        mod.get_axon_ntff_profile_hook = lambda: _hook[0]
        sys.modules["antenv.axon_hooks"] = mod
        antenv.axon_hooks = mod
        from trn_agent_boot.trn_boot import _ntff_profile_via_ctypes

        mod.set_axon_ntff_profile_hook(
            _ntff_profile_via_ctypes("/opt/axon/libaxon_pjrt.so")
        )
    except Exception as e:  # pragma: no cover - profiling is optional
        print(f"NTFF hook registration failed ({e}); tracing disabled")


def kernel(x, Alpha, M, Z, rv):
    x = np.ascontiguousarray(np.asarray(x, dtype=np.float32))
    Alpha = np.ascontiguousarray(np.asarray(Alpha, dtype=np.float32))
    M = np.ascontiguousarray(np.asarray(M, dtype=np.float32))
    Z = np.ascontiguousarray(np.asarray(Z, dtype=np.float32))
    rv = np.asarray(rv, dtype=np.float32)

    trace = bool(int(os.environ.get("BASS_KERNEL_TRACE", "0")))
    if trace:
        _ensure_ntff_hook()

    nc = build_program(rv)

    IS = C // N_CORES
    in_maps = []
    for c in range(N_CORES):
        im = {
            "x": np.ascontiguousarray(x[c * B_CORE : (c + 1) * B_CORE]),
            "Alpha": Alpha,
        }
        if CC:
            im["Ms"] = np.ascontiguousarray(M[:, c * IS : (c + 1) * IS])
            im["Zs"] = np.ascontiguousarray(Z[:, :, c * IS : (c + 1) * IS])
        else:
            im["M"] = M
            im["Z"] = Z
        in_maps.append(im)

    res = run_bass_kernel_spmd(
        nc,
        in_maps,
        core_ids=list(range(N_CORES)),
        trace=trace,
    )
    out = np.concatenate(
        [res.results[c]["out"] for c in range(N_CORES)], axis=0
    ).astype(np.float32)
    if trace:
        kernel.last_results = res
    return out


# revision 76
# speedup vs baseline: 1.1907x; 1.0077x over previous
"""Trainium2 Bass kernel for BinarizeConv2dSDP.

Math (reference):
    s   = M + rv @ Z          (the rsqrt normalization is sign-preserving:
                               w = (m + rv@z) * rsqrt(...) with rsqrt > 0,
                               so sign(w) == sign(s))
    bw  = sign(s)             (O, I, 3, 3)
    ba  = sign(x)             (B, C, H, W)
    out = conv2d(ba, bw, stride 1, pad 1) * Alpha

Strategy:
    - Data-parallel over batch: 8 cores x 4 images each. M/Z/Alpha replicated.
    - Weight synthesis on the PE: per 288-col chunk, one contiguous PSUM
      accumulation group [I @ M, then rv_k I @ Z_k x5] of fp32r matmuls at
      full rate (N >= 256); ACT signs straight from PSUM, PE transposes the
      signed taps into lhsT layout (lag-one behind the group signs), DVE
      packs to fp8. Only Z sees fp32r input rounding -> a handful of
      borderline sign flips, well under the 2e-2 gate.
      HW rules learned the hard way: accumulation groups must stay
      contiguous on the PE queue (no other matmul interleaved), and an
      engine write to PSUM followed by matmul accumulation races; only
      standalone LDWEIGHTS may fill arrival gaps (HAM clock-gate warm-up,
      along with filler matmuls before the stream).
    - Head DMAs on one sync queue, few big transfers (the trigger rate
      otherwise starves the wire; a second queue halves per-queue rate):
      x0's first row-chunk leads to absorb the queue's slow ramp, then M,
      Z as L(864)/R(288) column halves so chunk groups c0-c2 finish while
      the R half streams, then the rest of x. Image-0 sign chunks slot
      between the weight signs on ACT's FIFO.
    - Binarized conv, 5 passes per 8-row tile (was 6): sign(x) lives in a
      zero-padded [128, 115 x 64] fp8 image; rows 59..114 hold a one-column-
      left-shifted duplicate of rows 2..57. Vertical tap pairs (ky 0+1) use
      DoubleRow with rhs pair step 64 and lhsT pair stride 3*O; the
      (2,0)+(2,1) pair uses DoubleRow into the duplicate (rhs pair step
      57*64); (2,2) is a single matmul. +-1 is exact in fp8/bf16 and PSUM
      accumulates f32 -> integer-exact.
    - Evacuation (x Alpha) on DVE; output stored fp16 (conv integers
      <= 1152 are fp16-exact; only the Alpha product rounds, ~5e-4), host
      upcasts. Stores alternate sync/gpsimd queues, tail on sync.
    - BASS_KERNEL_CC=1 enables a sharded-synthesis variant (each core
      synthesizes 16 in-channels, AllGather rebuilds the lhsT): correct,
      but the collective eats the full SPMD core-launch skew (~50us on
      this stack), so it stays off.
"""

import os
import numpy as np

import concourse.bass as bass
import concourse.tile as tile
from concourse import bacc, mybir
from concourse.bass_utils import run_bass_kernel_spmd
from concourse.masks import make_identity

F32 = mybir.dt.float32
F32R = mybir.dt.float32r
F16 = mybir.dt.float16
BF16 = mybir.dt.bfloat16
FP8 = mybir.dt.float8e4

USE_FP8 = bool(int(os.environ.get("BASS_KERNEL_FP8", "1")))
OUT16 = bool(int(os.environ.get("BASS_KERNEL_OUT16", "1")))
N_WARM = int(os.environ.get("BASS_KERNEL_WARM", "12"))
# sharded weight synthesis + AllGather: correct, but the collective pays
# the full SPMD core-launch skew (~50us on this stack) -> off by default
CC = bool(int(os.environ.get("BASS_KERNEL_CC", "0"))) and USE_FP8

B_FULL = 32
N_CORES = 8
B_CORE = B_FULL // N_CORES  # 4 images per core
C = 128      # in channels
O = 128      # out channels
H = W = 56
HP = 58                      # padded rows
WP = 64 if USE_FP8 else 58   # padded row stride (64 -> vertical pair step 64B)
DUP = 57                     # dup row r stored at padded row r + DUP
HP2 = HP + DUP               # 114 rows total (2..57 duplicated, shifted left 1)
KS = 3
NTAPS = KS * KS
IKK = C * NTAPS  # 1152
ROWS_PER_TILE = 8           # output rows per PSUM tile -> N = 8*56 = 448
N_TILE = ROWS_PER_TILE * W  # 448 fp32 <= 512 (one PSUM bank)
N_ROW_TILES = H // ROWS_PER_TILE  # 7
ADT = FP8 if USE_FP8 else BF16
ODT = F16 if OUT16 else F32

# synth column chunks: 4 x 288, rotating over two PSUM banks; N >= 256
# keeps the f32r matmuls at full rate
SYN_CHUNKS = tuple((c, c + 288) for c in range(0, IKK, 288))
# x0 arrives in row chunks so sign/conv can start before the full image
X0_CHUNKS = ((0, 16), (16, 36), (36, 56))
XN_CHUNKS = ((0, 28), (28, 56))


def build_program(rv: np.ndarray, n_img: int = B_CORE):
    """Build the per-core Bass program. rv values are baked as immediates."""
    nc = bacc.Bacc(
        "TRN2",
        target_bir_lowering=False,
        debug=False,
        num_devices=N_CORES,
    )

    x_t = nc.dram_tensor("x", (n_img, C, H, W), F32, kind="ExternalInput").ap()
    a_t = nc.dram_tensor("Alpha", (O, 1, 1), F32, kind="ExternalInput").ap()
    IS = C // N_CORES  # in-channels synthesized per core in the CC path
    if CC:
        m_t = nc.dram_tensor(
            "Ms", (O, IS, KS, KS), F32R, kind="ExternalInput"
        ).ap()
        z_t = nc.dram_tensor(
            "Zs", (5, O, IS, KS, KS), F32R, kind="ExternalInput"
        ).ap()
        bw_stage = nc.dram_tensor("bw_stage", (IS, NTAPS * O), FP8)
        bw_all = nc.dram_tensor(
            "bw_all", (C, NTAPS * O), FP8, addr_space="Shared"
        )
    else:
        m_t = nc.dram_tensor(
            "M", (O, C, KS, KS), F32R, kind="ExternalInput"
        ).ap()
        z_t = nc.dram_tensor(
            "Z", (5, O, C, KS, KS), F32R, kind="ExternalInput"
        ).ap()
    out_t = nc.dram_tensor("out", (n_img, O, H, W), ODT, kind="ExternalOutput").ap()

    rv = np.asarray(rv, dtype=np.float32).reshape(-1)
    assert rv.shape[0] == 5

    with tile.TileContext(nc) as tc:
        with (
            tc.tile_pool(name="const", bufs=1) as const_pool,
            tc.tile_pool(name="wsyn", bufs=1) as wsyn_pool,
            tc.tile_pool(name="imgs", bufs=1) as img_pool,
            tc.tile_pool(name="xstage", bufs=4) as x_pool,
            tc.tile_pool(name="evac", bufs=14) as ev_pool,
            tc.tile_pool(name="cpsum", bufs=4, space="PSUM") as cpsum_pool,
            tc.tile_pool(name="spsum", bufs=1, space="PSUM") as spsum_pool,
            tc.tile_pool(name="tpsum", bufs=1, space="PSUM") as tpsum_pool,
        ):
            # --- constants first: warm rhs + rv_k identities build on
            # gpsimd while the first DMAs stream.
            warm_rhs = const_pool.tile([128, 448], BF16)
            nc.gpsimd.memset(warm_rhs, 0.0)
            identity = const_pool.tile([128, 128], BF16)
            make_identity(nc, identity)
            rvI_f32s = {}
            for k in (5, 0, 1, 2, 3, 4):
                # k=5 is the plain identity (1.0): stationary for the
                # M-initializing matmul of each synthesis group — built
                # first so its f32r copy can lead the ACT FIFO
                t32 = const_pool.tile([128, 128], F32, name=f"rvI_f32_{k}")
                nc.gpsimd.memset(t32, 0.0)
                nc.gpsimd.affine_select(
                    out=t32,
                    in_=t32,
                    compare_op=mybir.AluOpType.not_equal,
                    fill=float(rv[k]) if k < 5 else 1.0,
                    base=0,
                    pattern=[[-1, 128]],
                    channel_multiplier=1,
                )
                rvI_f32s[k] = t32

            # --- head DMAs on TWO HWDGE queues (sync + scalar): a single
            # queue only sustains ~300 GB/s, two reach the fabric limit.
            # Per-queue FIFO order == priority: Z in synthesis order, then
            # x0 row chunks, then x1..x3.
            if CC:
                m_sb = wsyn_pool.tile([O, IS * NTAPS], F32R)
                zs_sb = wsyn_pool.tile([O, 5 * IS * NTAPS], F32R)
            else:
                m_sb = wsyn_pool.tile([O, IKK], F32R)
                z_sbs = [
                    wsyn_pool.tile([O, IKK], F32R, name=f"z{k}", tag=f"z{k}")
                    for k in range(5)
                ]
            x_tiles = [
                x_pool.tile([C, H * W], F32, name=f"x{img}", tag="xin")
                for img in range(n_img)
            ]

            def xdma(img, r0, r1):
                xv = x_tiles[img].rearrange("c (h w) -> c h w", w=W)
                return (xv[:, r0:r1, :], x_t[img, :, r0:r1, :])

            alpha_sb = const_pool.tile([O, 1], F32)
            nc.scalar.dma_start(alpha_sb, a_t.rearrange("o a b -> o (a b)"))
            # Z streams in chunk-major order (cc outer, k inner) so each
            # synthesis group's operands land back-to-back and the group
            # completes during the stream. Chunks alternate between the
            # sync and vector trigger engines: one queue only reaches
            # ~300 GB/s, two saturate the fabric, and per-queue FIFO still
            # preserves the chunk-major arrival order.
            # x0's first chunk leads: it absorbs the DMA queue's slow ramp
            # (~2us at <200 GB/s) with non-critical bytes.
            nc.sync.dma_start(*xdma(0, *X0_CHUNKS[0]))
            if CC:
                nc.sync.dma_start(
                    m_sb, m_t.rearrange("o i kh kw -> o (i kh kw)")
                )
                nc.sync.dma_start(
                    zs_sb.rearrange("o (k n) -> o k n", k=5),
                    z_t.rearrange("k o i kh kw -> o k (i kh kw)"),
                )
            else:
                # balanced split across the two HWDGE queues (~215 GB/s
                # each): sync carries M + even-k Z halves, scalar carries
                # odd-k + z4 (the synthesis tail), so z4L lands ~4us
                # earlier than a single queue manages.
                nc.sync.dma_start(
                    m_sb, m_t.rearrange("o i kh kw -> o (i kh kw)")
                )
                ZL = slice(0, 3 * 288)
                ZR = slice(3 * 288, IKK)

                def zdma(eng, k, half):
                    eng.dma_start(
                        z_sbs[k][:, half],
                        z_t[k].rearrange("o i kh kw -> o (i kh kw)")[:, half],
                    )

                # scalar-queue triggers interleave with the rvI f32r
                # copies on the ACT FIFO; rvI[5]/rvI[0] lead (they gate
                # chunk group c0's first matmuls)
                rvI_t = {}

                def rvi_copy(k):
                    t = const_pool.tile([128, 128], F32R, name=f"rvI{k}")
                    nc.scalar.copy(t, rvI_f32s[k])
                    rvI_t[k] = t

                rvi_copy(5)
                zdma(nc.scalar, 1, ZL)
                rvi_copy(0)
                zdma(nc.scalar, 3, ZL)
                rvi_copy(1)
                zdma(nc.scalar, 4, ZL)
                rvi_copy(2)
                zdma(nc.scalar, 1, ZR)
                rvi_copy(3)
                zdma(nc.scalar, 3, ZR)
                rvi_copy(4)
                zdma(nc.scalar, 4, ZR)
                for k in (0, 2):
                    zdma(nc.sync, k, ZL)
                for k in (0, 2):
                    zdma(nc.sync, k, ZR)
                rvI = [rvI_t[k] for k in range(6)]
            for r0, r1 in X0_CHUNKS[1:]:
                nc.sync.dma_start(*xdma(0, r0, r1))
            for img in range(1, n_img):
                nc.sync.dma_start(
                    x_tiles[img], x_t[img].rearrange("c h w -> c (h w)")
                )
            if CC:
                rvI = []
                for k in range(6):
                    t = const_pool.tile([128, 128], F32R, name=f"rvI{k}")
                    nc.scalar.copy(t, rvI_f32s[k])
                    rvI.append(t)

            # --- PE warm-up: keep the HAM clock gate ramping while the
            # head DMAs stream, so neither synth nor conv starts cold.
            warm_ps = cpsum_pool.tile([O, N_TILE], F32, tag="cv")
            for _ in range(N_WARM):
                nc.tensor.matmul(
                    warm_ps, identity, warm_rhs, start=True, stop=True
                )

            # --- per-image padded sign(x) buffers (borders zeroed once) ---
            padded = []
            for img in range(n_img):
                pd = img_pool.tile(
                    [C, HP2 * WP], ADT, name=f"pad{img}", tag=f"pad{img}"
                )
                pd3 = pd.rearrange("p (h w) -> p h w", w=WP)
                nc.gpsimd.memset(pd3[:, 0, 0:HP], 0.0)
                nc.gpsimd.memset(pd3[:, HP - 1, 0:HP], 0.0)
                nc.gpsimd.memset(pd3[:, 1 : HP - 1, 0:1], 0.0)
                nc.gpsimd.memset(pd3[:, 1 : HP - 1, HP - 1 : HP], 0.0)
                # dup of bottom-pad row 57 (zero); dup cols >= 56 are unread
                nc.gpsimd.memset(pd3[:, HP2 - 1, 0:HP], 0.0)
                padded.append(pd3)

            # --- weight synthesis on PE: I @ M starts each PSUM group,
            # then 5 f32r matmuls accumulate rv_k Z_k, sign from PSUM.
            # bwg is the unified conv weight tile [C, (tap, O)].
            bwg = wsyn_pool.tile([C, NTAPS * O], ADT)
            bwg3 = bwg.rearrange("p (t o) -> p t o", o=O)
            if CC:
                bw_nat = wsyn_pool.tile([O, IS * NTAPS], BF16)
                bw3 = bw_nat.rearrange("o (i t) -> o i t", t=NTAPS)
                bw_small = wsyn_pool.tile([IS, NTAPS * O], FP8)
                syn_s = spsum_pool.tile([O, IS * NTAPS], F32)
                tpXa = tpsum_pool.tile([IS, 4 * O], BF16)
                tpXb = tpsum_pool.tile([IS, 5 * O], BF16)
            else:
                bw_nat = wsyn_pool.tile([O, IKK], BF16)
                bw3 = bw_nat.rearrange("o (i t) -> o i t", t=NTAPS)
                syn = [
                    spsum_pool.tile([O, 288], F32, name=f"syn{i}")
                    for i in range(2)
                ]
                tpP = tpsum_pool.tile([128, 4 * O], BF16)
                tpS = tpsum_pool.tile([128, 5 * O], BF16)

            def transpose_chunk(ic):
                psl = slice(ic * 32, (ic + 1) * 32)
                for t in range(NTAPS):
                    dst, toff = (tpP, t * O) if t < 4 else (tpS, (t - 4) * O)
                    nc.tensor.transpose(
                        dst[psl, toff : toff + O],
                        bw3[:, psl, t],
                        identity,
                        tile_position=(0, ic * 32),
                    )

            # --- signs: main rows + shifted dup rows, chunked + interleaved
            # on ACT so conv tiles release as x rows land.
            def sign_main_rows(img, r0, r1):
                # pd rows 1+r0 .. 1+r1 <- sign(x rows r0..r1)
                pd3 = padded[img]
                xi = x_tiles[img].rearrange("c (h w) -> c h w", w=W)
                nc.scalar.sign(
                    pd3[:, 1 + r0 : 1 + r1, 1 : 1 + W], xi[:, r0:r1, :]
                )

            def sign_dup_rows(img, r0, r1):
                # dup rows r (=pd row DUP+r), r in [r0,r1) subset of [2,57):
                # dup[r][c] = pd[r][c+1] = sign(x[r-1][c]), c < 56
                pd3 = padded[img]
                xi = x_tiles[img].rearrange("c (h w) -> c h w", w=W)
                nc.scalar.sign(
                    pd3[:, DUP + r0 : DUP + r1, 0:W],
                    xi[:, r0 - 1 : r1 - 1, :],
                )

            def sign_image_chunk(img, r0, r1):
                sign_main_rows(img, r0, r1)
                sign_dup_rows(img, max(2, r0 + 1), min(57, r1 + 1))

            def sign_image(img):
                for r0, r1 in XN_CHUNKS:
                    sign_image_chunk(img, r0, r1)

            if CC:
                # sharded synthesis: this core owns IS in-channels; one
                # PSUM group (I @ Ms + sum rv_k Zs_k), sign, transpose to
                # [IS, (t, o)], pack fp8, AllGather the 8 slices into the
                # full lhsT and load it back.
                sign_image_chunk(0, *X0_CHUNKS[0])
                nc.tensor.matmul(
                    syn_s, rvI[5], m_sb, start=True, stop=False
                )
                NS = IS * NTAPS
                for k in range(5):
                    nc.tensor.matmul(
                        syn_s,
                        rvI[k],
                        zs_sb[:, k * NS : (k + 1) * NS],
                        start=False,
                        stop=(k == 4),
                    )
                nc.scalar.sign(bw_nat, syn_s)
                for t in range(NTAPS):
                    dst, toff = (
                        (tpXa, t * O) if t < 4 else (tpXb, (t - 4) * O)
                    )
                    nc.tensor.transpose(
                        dst[:, toff : toff + O], bw3[:, :, t], identity
                    )
                nc.vector.tensor_copy(bw_small[:, : 4 * O], tpXa)
                nc.vector.tensor_copy(bw_small[:, 4 * O :], tpXb)
                nc.sync.dma_start(bw_stage.ap(), bw_small)
                nc.gpsimd.collective_compute(
                    "AllGather",
                    mybir.AluOpType.bypass,
                    replica_groups=[list(range(N_CORES))],
                    ins=[bw_stage.ap().opt()],
                    outs=[bw_all.ap().opt()],
                )
                nc.sync.dma_start(bwg, bw_all.ap())
                sign_image_chunk(0, *X0_CHUNKS[1])
                sign_image_chunk(0, *X0_CHUNKS[2])
            else:
                # c-major: each chunk's accumulation group stays contiguous
                # on the PE queue (interleaving open matmul groups corrupts
                # on HW). The M term enters as the group's start matmul
                # (I @ M) — an engine write into PSUM followed by matmul
                # accumulation races on HW. Standalone LDWEIGHTS between
                # the arrival-paced matmuls of chunk 0 keep the HAM
                # activity window busy; chunk ic's transposes are emitted
                # after group ic+1 (lag-one) so their sign has retired.
                for cc, (c0, c1) in enumerate(SYN_CHUNKS):
                    ps = syn[cc % 2]
                    nc.tensor.matmul(
                        ps, rvI[5], m_sb[:, c0:c1], start=True, stop=False
                    )
                    for k in range(5):
                        nc.tensor.matmul(
                            ps,
                            rvI[k],
                            z_sbs[k][:, c0:c1],
                            start=False,
                            stop=(k == 4),
                        )
                        if cc == 0 and k < 4:
                            for _ in range(5):
                                nc.tensor.ldweights(warm_rhs[:, 0:128])
                    nc.scalar.sign(bw_nat[:, c0:c1], ps)
                    if cc >= 1:
                        transpose_chunk(cc - 1)
                        sign_image_chunk(0, *X0_CHUNKS[cc - 1])
                transpose_chunk(3)
                nc.vector.tensor_copy(
                    bwg3[:, 0:4, :], tpP.rearrange("p (t o) -> p t o", o=O)
                )
                nc.vector.tensor_copy(
                    bwg3[:, 4:NTAPS, :],
                    tpS.rearrange("p (t o) -> p t o", o=O),
                )

            # --- main conv loop; next image's sign emitted before this
            # image's tiles so ACT never head-of-line blocks ---
            for img in range(n_img):
                if img + 1 < n_img:
                    sign_image(img + 1)
                pd3 = padded[img]

                for nt in range(N_ROW_TILES):
                    y0 = nt * ROWS_PER_TILE
                    cv = cpsum_pool.tile([O, N_TILE], F32, tag="cv")
                    if USE_FP8:
                        # 3 vertical pairs {(0,kx),(1,kx)}: rhs pair step
                        # WP, lhsT pair = taps (kx, kx+3) at stride 3*O
                        for kx in range(KS):
                            win0 = pd3[:, y0 : y0 + ROWS_PER_TILE, kx : kx + W]
                            ap4 = bass.AP(
                                win0.tensor,
                                win0.offset,
                                [list(win0.ap[0]), [WP, 2]]
                                + [list(p) for p in win0.ap[1:]],
                            )
                            wv = bwg3[:, kx, :]
                            apW = bass.AP(
                                wv.tensor,
                                wv.offset,
                                [list(wv.ap[0]), [3 * O, 2], [1, O]],
                            )
                            nc.tensor.matmul(
                                cv,
                                apW,
                                ap4,
                                start=(kx == 0),
                                stop=False,
                                perf_mode=mybir.MatmulPerfMode.DoubleRow,
                            )
                        # pair {(2,0),(2,1)}: elem 1 in the shifted dup rows
                        winD = pd3[:, y0 + 2 : y0 + 2 + ROWS_PER_TILE, 0:W]
                        apD = bass.AP(
                            winD.tensor,
                            winD.offset,
                            [list(winD.ap[0]), [DUP * WP, 2]]
                            + [list(p) for p in winD.ap[1:]],
                        )
                        nc.tensor.matmul(
                            cv,
                            bwg3[:, 6:8, :],
                            apD,
                            start=False,
                            stop=False,
                            perf_mode=mybir.MatmulPerfMode.DoubleRow,
                        )
                        # single tap (2,2)
                        winS = pd3[
                            :, y0 + 2 : y0 + 2 + ROWS_PER_TILE, 2 : 2 + W
                        ]
                        nc.tensor.matmul(
                            cv, bwg3[:, 8, :], winS, start=False, stop=True
                        )
                    else:
                        t = 0
                        for ky in range(KS):
                            for kx in range(KS):
                                win = pd3[
                                    :,
                                    y0 + ky : y0 + ky + ROWS_PER_TILE,
                                    kx : kx + W,
                                ]
                                nc.tensor.matmul(
                                    cv,
                                    bwg3[:, t, :],
                                    win,
                                    start=(t == 0),
                                    stop=(t == NTAPS - 1),
                                )
                                t += 1
                    ev = ev_pool.tile([O, N_TILE], ODT, tag="ev")
                    nc.vector.tensor_scalar_mul(ev, cv, alpha_sb[:, 0:1])
                    # alternate store queues; keep the final stores on the
                    # low-latency HWDGE path so the tail drains fast
                    tile_n = img * N_ROW_TILES + nt
                    eng = (
                        nc.sync
                        if (tile_n % 2 == 0 or tile_n >= 26)
                        else nc.gpsimd
                    )
                    eng.dma_start(
                        out_t[img, :, y0 : y0 + ROWS_PER_TILE, :],
                        ev.rearrange("o (h w) -> o h w", w=W),
                    )

    nc.compile()
    return nc


def _ensure_ntff_hook():
    """Register the axon NTFF profiling hook if the image's antenv lacks it.

    Only used when BASS_KERNEL_TRACE=1 (dev profiling); best-effort.
    """
    import sys
    import types

    try:
        import antenv

        if hasattr(antenv, "axon_hooks"):
            return
        mod = types.ModuleType("antenv.axon_hooks")
        _hook = [None]
        mod.set_axon_ntff_profile_hook = lambda h: _hook.__setitem__(0, h)
        mod.get_axon_ntff_profile_hook = lambda: _hook[0]
        sys.modules["antenv.axon_hooks"] = mod
        antenv.axon_hooks = mod
        from trn_agent_boot.trn_boot import _ntff_profile_via_ctypes

        mod.set_axon_ntff_profile_hook(
            _ntff_profile_via_ctypes("/opt/axon/libaxon_pjrt.so")
        )
    except Exception as e:  # pragma: no cover - profiling is optional
        print(f"NTFF hook registration failed ({e}); tracing disabled")


def kernel(x, Alpha, M, Z, rv):
    x = np.ascontiguousarray(np.asarray(x, dtype=np.float32))
    Alpha = np.ascontiguousarray(np.asarray(Alpha, dtype=np.float32))
    M = np.ascontiguousarray(np.asarray(M, dtype=np.float32))
    Z = np.ascontiguousarray(np.asarray(Z, dtype=np.float32))
    rv = np.asarray(rv, dtype=np.float32)

    trace = bool(int(os.environ.get("BASS_KERNEL_TRACE", "0")))
    if trace:
        _ensure_ntff_hook()

    nc = build_program(rv)

    IS = C // N_CORES
    in_maps = []
    for c in range(N_CORES):
        im = {
            "x": np.ascontiguousarray(x[c * B_CORE : (c + 1) * B_CORE]),
            "Alpha": Alpha,
        }
        if CC:
            im["Ms"] = np.ascontiguousarray(M[:, c * IS : (c + 1) * IS])
            im["Zs"] = np.ascontiguousarray(Z[:, :, c * IS : (c + 1) * IS])
        else:
            im["M"] = M
            im["Z"] = Z
        in_maps.append(im)

    res = run_bass_kernel_spmd(
        nc,
        in_maps,
        core_ids=list(range(N_CORES)),
        trace=trace,
    )
    out = np.concatenate(
        [res.results[c]["out"] for c in range(N_CORES)], axis=0
    ).astype(np.float32)
    if trace:
        kernel.last_results = res
    return out
